# revision 10
# baseline (speedup 1.0000x reference)
"""Trainium2 Bass kernel for CosineAttention:

    out = sigmoid((xn @ xn.T) @ x)   where xn = x / ||x_row||

Key algebraic optimization: reassociate (xn @ xn.T) @ x = xn @ (xn.T @ x).
G = xn.T @ x is [D, D] — the O(N^2 D) similarity matrix is never formed.
Total work drops from ~275 GFLOP to ~34 GFLOP.

Sharding: rows of x across 8 cores. Each core:
  1. loads its [N/8, D] row block, computes row norms + normalized rows
  2. computes partial G_c = xn_c.T @ x_c  (f32 PSUM accumulation)
  3. AllReduce(G) across the 8 cores (2 column chunks, overlapped)
  4. out_c = sigmoid(xn_c @ G)
The host concatenates the 8 row blocks.

Precision (bf16 config): G's diagonal (~256) dwarfs its off-diagonal
entries (~3), so bf16 rounding of G would put ~0.5 absolute error on the
diagonal. We instead compute G' = G - c*I in mm1 (by accumulating
-c * shifted-identity into the PSUM), AllReduce/round G' (small entries,
small absolute error), and add the c*xn term back exactly in f32 before
the sigmoid: z = xn_bf @ G'_bf + c*xn.

Schedule:
  - tiny warmup AllReduce at t~0 absorbs the first-collective
    barrier/launch-skew window under the compute phase
  - mm1 is row-tile-outer so PE starts as soon as tile 0 is loaded
  - G AllReduce is split into two column halves; mm2 on half 0 overlaps
    the half-1 reduce
  - xn.T built by DMA transpose (bf16) or PE transpose (f32r config)
"""

import numpy as np

import concourse.bass as bass  # noqa: F401
import concourse.mybir as mybir
import concourse.tile as tile
from concourse import bacc
from concourse.bass_utils import run_bass_kernel_spmd
from concourse.masks import make_identity

F32 = mybir.dt.float32
F32R = mybir.dt.float32r
BF16 = mybir.dt.bfloat16
F16 = mybir.dt.float16
AFT = mybir.ActivationFunctionType

N, D = 8192, 1024
NCORES = 8
R = N // NCORES  # rows per core
P = 128
RT = R // P      # row tiles per core
KT = D // P      # contraction tiles (mm2) / G row tiles
FD = 512         # matmul moving free dim (one PSUM bank of f32)
NH = D // FD     # column halves
GROUPS = [list(range(NCORES))]
DIAG_C = 256.0   # ~mean of diag(G); exact in bf16


def _emit_body(tc, xb, out, mm_dt, ar_dt, ctx, use_diag=True):
    nc = tc.nc
    xb_t = xb.rearrange("(rt p) d -> rt p d", p=P)
    out_t = out.rearrange("(rt p) d -> rt p d", p=P)
    f32r_mode = mm_dt == F32R
    diag_trick = mm_dt in (BF16, F16) and use_diag

    persist = ctx.enter_context(tc.tile_pool(name="persist", bufs=1))
    load = ctx.enter_context(tc.tile_pool(name="load", bufs=3))
    small = ctx.enter_context(tc.tile_pool(name="small", bufs=1))
    gloc = ctx.enter_context(tc.tile_pool(name="gloc", bufs=3))
    gstage = ctx.enter_context(tc.tile_pool(name="gstage", bufs=3))
    ostage = ctx.enter_context(tc.tile_pool(name="ostage", bufs=3))
    ps = ctx.enter_context(tc.tile_pool(name="ps", bufs=1, space="PSUM"))
    dram = ctx.enter_context(tc.tile_pool(name="dram", bufs=1, space="DRAM"))

    # ---- warmup collective: absorbs first-collective barrier/skew.
    # Input is uninitialized garbage — output is unused; the point is to
    # get this core's first CC doorbell written as early as possible.
    w_in = dram.tile([P, 4], F32, tag="w_in")
    w_out = dram.tile([P, 4], F32, tag="w_out")
    nc.gpsimd.collective_compute(
        "AllReduce", mybir.AluOpType.add, replica_groups=GROUPS,
        ins=[w_in.opt()], outs=[w_out.opt()],
    )

    if f32r_mode:
        ident = persist.tile([P, P], F32, tag="ident")
        make_identity(nc, ident)
    if diag_trick:
        identb = persist.tile([P, P], mm_dt, tag="identb")
        make_identity(nc, identb)
        # dsh[s]: [P, FD] bf16, -c * identity placed at columns [s*128,(s+1)*128)
        dsh = []
        for s in range(FD // P):
            t_dsh = persist.tile([P, FD], mm_dt, tag=f"dsh{s}", name=f"dsh{s}")
            nc.vector.memset(t_dsh, 0.0)
            nc.scalar.mul(t_dsh[:, s * P:(s + 1) * P], identb, -DIAG_C / NCORES)
            dsh.append(t_dsh)

    # ---- phase 0: load row block, norms, casts ----
    xbr, xnr, cxn = [], [], []
    for rt in range(RT):
        xf = load.tile([P, D], F32, tag="xf")
        nc.sync.dma_start(out=xf, in_=xb_t[rt])
        t_xbr = persist.tile([P, D], mm_dt, tag=f"xbr{rt}")
        nc.vector.tensor_copy(out=t_xbr, in_=xf)
        sq = load.tile([P, D], BF16, tag="sq")
        ss = small.tile([P, 1], F32, tag=f"ss{rt}")
        nc.scalar.activation(out=sq, in_=xf, func=AFT.Square, accum_out=ss)
        nrm = small.tile([P, 1], F32, tag=f"nrm{rt}")
        nc.scalar.sqrt(nrm, ss)
        rn = small.tile([P, 1], F32, tag=f"rn{rt}")
        nc.vector.reciprocal(rn, nrm)
        t_xnr = persist.tile([P, D], mm_dt, tag=f"xnr{rt}")
        nc.vector.tensor_scalar_mul(t_xnr, xf, rn)
        xbr.append(t_xbr)
        xnr.append(t_xnr)
        if diag_trick:
            rc = small.tile([P, 1], F32, tag=f"rc{rt}")
            nc.scalar.mul(rc, rn, DIAG_C)
            t_cxn = persist.tile([P, D], F32, tag=f"cxn{rt}")
            nc.vector.tensor_scalar_mul(t_cxn, xf, rc)
            cxn.append(t_cxn)

    # ---- phase 1: G'_c = xn_c.T @ x_c (- c*I), wave per column half ----
    g_in, g_out = [], []
    for nh in range(NH):
        g_in.append(dram.tile([D, FD], ar_dt, tag=f"g_in{nh}", name=f"g_in{nh}"))
        g_out.append(dram.tile([D, FD], ar_dt, tag=f"g_out{nh}", name=f"g_out{nh}"))
    for nh in range(NH):
        psg = [ps.tile([P, FD], F32, tag=f"acc{mt}", name=f"psg{nh}_{mt}")
               for mt in range(KT)]
        for rt in range(RT):
            for mt in range(KT):
                has_diag = diag_trick and (mt // (FD // P) == nh)
                nc.tensor.matmul(
                    psg[mt],
                    lhsT=xnr[rt][:, mt * P:(mt + 1) * P],
                    rhs=xbr[rt][:, nh * FD:(nh + 1) * FD],
                    start=(rt == 0),
                    stop=(rt == RT - 1) and not has_diag,
                )
        if diag_trick:
            # diag-containing tiles get one extra matmul: += -c * shifted I
            for mt in range(KT):
                if mt // (FD // P) == nh:
                    nc.tensor.matmul(
                        psg[mt], lhsT=identb, rhs=dsh[mt % (FD // P)],
                        start=False, stop=True,
                    )
        g_in_t = g_in[nh].rearrange("(mt p) f -> mt p f", p=P)
        for mt in range(KT):
            gl = gloc.tile([P, FD], ar_dt, tag="gloc")
            nc.vector.tensor_copy(out=gl, in_=psg[mt])
            nc.sync.dma_start(out=g_in_t[mt], in_=gl)
        # chunked AllReduce: half nh reduces while the other half computes
        nc.gpsimd.collective_compute(
            "AllReduce", mybir.AluOpType.add, replica_groups=GROUPS,
            ins=[g_in[nh].opt()], outs=[g_out[nh].opt()],
        )

    # ---- phase 1.5: xnT (DMA transpose for bf16, PE transpose for f32r) ----
    xnT = []
    for kt in range(KT):
        t_xnT = persist.tile([P, D], mm_dt, tag=f"xnT{kt}")
        for rt in range(RT):
            src = xnr[rt][:, kt * P:(kt + 1) * P]
            if f32r_mode:
                tpt = ps.tile([P, P], F32, tag=f"acc{rt % 2}", name=f"tp{kt}_{rt}")
                nc.tensor.transpose(tpt, src.bitcast(F32), ident)
                nc.vector.tensor_copy(out=t_xnT[:, rt * P:(rt + 1) * P], in_=tpt)
            else:
                nc.sync.dma_start_transpose(
                    out=t_xnT[:, rt * P:(rt + 1) * P], in_=src
                )
        xnT.append(t_xnT)

    # ---- phases 3+4 per column half: load G half (round), mm2, sigmoid ----
    for nh in range(NH):
        g_out_t = g_out[nh].rearrange("(kt p) f -> kt p f", p=P)
        gr = []
        for kt in range(KT):
            t_gr = persist.tile([P, FD], mm_dt, tag=f"gr{nh}_{kt}")
            if f32r_mode:
                gs = gstage.tile([P, FD], F32, tag="gs")
                nc.sync.dma_start(out=gs, in_=g_out_t[kt])
                nc.vector.tensor_copy(out=t_gr, in_=gs)
            else:
                nc.sync.dma_start(out=t_gr, in_=g_out_t[kt])
            gr.append(t_gr)
        for mt in range(RT):
            ps_z = ps.tile([P, FD], F32, tag=f"acc{mt}", name=f"psz{nh}_{mt}")
            for kt in range(KT):
                nc.tensor.matmul(
                    ps_z,
                    lhsT=xnT[kt][:, mt * P:(mt + 1) * P],
                    rhs=gr[kt],
                    start=(kt == 0),
                    stop=(kt == KT - 1),
                )
            if diag_trick:
                nc.vector.tensor_add(
                    ps_z, ps_z, cxn[mt][:, nh * FD:(nh + 1) * FD]
                )
            ob = ostage.tile([P, FD], F32, tag="ob")
            nc.scalar.activation(out=ob, in_=ps_z, func=AFT.Sigmoid)
            nc.sync.dma_start(out=out_t[mt][:, nh * FD:(nh + 1) * FD], in_=ob)


def build(mm_dt=F16, ar_dt=F16, use_diag=True):
    from contextlib import ExitStack

    nc = bacc.Bacc("TRN2", target_bir_lowering=False, debug=False,
                   num_devices=NCORES)
    xb = nc.dram_tensor("xb", [R, D], F32, kind="ExternalInput").ap()
    out = nc.dram_tensor("out", [R, D], F32, kind="ExternalOutput").ap()
    with tile.TileContext(nc) as tc:
        with ExitStack() as ctx:
            _emit_body(tc, xb, out, mm_dt, ar_dt, ctx, use_diag)
    nc.compile()
    return nc


_NC_CACHE = {}


def _get_nc(mm_dt=F16, ar_dt=F16):
    key = (str(mm_dt), str(ar_dt))
    if key not in _NC_CACHE:
        _NC_CACHE[key] = build(mm_dt, ar_dt)
    return _NC_CACHE[key]


def kernel(x: np.ndarray) -> np.ndarray:
    x = np.asarray(x, dtype=np.float32)
    assert x.shape == (N, D), x.shape
    nc = _get_nc()
    in_maps = [{"xb": x[c * R:(c + 1) * R]} for c in range(NCORES)]
    res = run_bass_kernel_spmd(nc, in_maps, list(range(NCORES)))
    return np.concatenate([res.results[c]["out"] for c in range(NCORES)], axis=0)
